# revision 11
# baseline (speedup 1.0000x reference)
"""Trainium2 Bass kernel for nn_ConvertParamsTEtoParams0TE.

Computation (per batch b, NH=64, NV=256):
    s       = sqrt(covh_diag1[b])                      # [64]
    inner   = covhTE1[b] @ wt1[b]                      # [64,256]
    wtTE2_b = 0.5*inner/s[:,None] + s[:,None]*wtTE1[b]
    bTE2_b  = bTE1[b] + muh1[b]@wtTE1[b] + muhTE1[b]@wt1[b]
    sig2TE passthrough.

Mapping: one 128x65 stationary x 128x256 moving matmul per batch computes the
whole thing.  Stationary rows 0-63 hold (covh * 0.5/s_i rows)^T, rows 64-127
hold diag(s); column 64 holds [muhTE1; muh1].  Moving rows 0-63 = wt1[b],
rows 64-127 = wtTE1[b].  PSUM rows 0-63 = wtTE2[b], row 64 = bTE2[b]-bTE1[b].

Sharding: pure data parallel over the batch dim, 8 NeuronCores x 512 batches.
"""

import numpy as np

import concourse.bacc as bacc
import concourse.mybir as mybir
import concourse.tile as tile
from concourse.bass_utils import run_bass_kernel_spmd
from concourse.masks import make_identity

B, NH, NV = 4096, 64, 256
N_CORES = 8
BS = B // N_CORES        # batches per core
GROUP = 16               # batches per inner group
NPAIR = GROUP // 2
N_GROUPS = BS // GROUP   # 64

USE_F32R = False         # float32r matmul: 4x faster PE, ~1e-4 rel err

_cached = {}


def _build_program():
    f32 = mybir.dt.float32
    mmdt = mybir.dt.float32r if USE_F32R else f32

    nc = bacc.Bacc("TRN2", target_bir_lowering=False)

    wtTE1_d = nc.declare_dram_parameter("wtTE1", [BS, NH, NV], mmdt, isOutput=False)
    wt1_d = nc.declare_dram_parameter("wt1", [BS, NH, NV], mmdt, isOutput=False)
    covh_d = nc.declare_dram_parameter("covhTE1", [BS, NH, NH], f32, isOutput=False)
    cd_d = nc.declare_dram_parameter("covh_diag1", [BS, NH], f32, isOutput=False)
    muh1_d = nc.declare_dram_parameter("muh1", [BS, NH], f32, isOutput=False)
    muhTE1_d = nc.declare_dram_parameter("muhTE1", [BS, NH], f32, isOutput=False)
    bTE1_d = nc.declare_dram_parameter("bTE1", [BS, NV], f32, isOutput=False)

    wtTE2_d = nc.declare_dram_parameter("wtTE2", [BS, NH, NV], f32, isOutput=True)
    bTE2_d = nc.declare_dram_parameter("bTE2", [BS, NV], f32, isOutput=True)

    with tile.TileContext(nc) as tc:
        with (
            tc.tile_pool(name="const", bufs=1) as const_pool,
            tc.tile_pool(name="mvp", bufs=3) as mv_pool,
            tc.tile_pool(name="statp", bufs=4) as stat_pool,
            tc.tile_pool(name="covhp", bufs=3) as covh_pool,
            tc.tile_pool(name="stagep", bufs=3) as stage_pool,
            tc.tile_pool(name="rowp", bufs=2) as row_pool,
            tc.tile_pool(name="prep", bufs=3) as prep_pool,
            tc.tile_pool(name="finp", bufs=2) as fin_pool,
            tc.tile_pool(name="dramp", bufs=1, space="DRAM") as dram_pool,
            tc.tile_pool(name="ps_mm", bufs=4, space="PSUM") as ps_mm_pool,
            tc.tile_pool(name="ps_tp", bufs=2, space="PSUM") as ps_tp_pool,
            tc.tile_pool(name="ps_prep", bufs=2, space="PSUM") as ps_prep_pool,
        ):
            ident = const_pool.tile([128, 128], f32)
            make_identity(nc, ident)

            scratch = dram_pool.tile([BS, NV], f32)

            c_pairs_cur = None   # [128, 64]: col j = [c_{2j}; c_{2j+1}], c=0.5/s
            s_stack_cur = None   # [128, 64]: rows 64-127 col b = s_b
            mu_cols_cur = None   # [128, 64]: col b = [muhTE1_b; muh1_b]

            for g in range(N_GROUPS):
                b0 = g * GROUP

                if g % (128 // GROUP) == 0:
                    # c-pipe for batches [b0, b0+128): c = 1/(2*sqrt(diag))
                    cc = b0
                    cd_pair = prep_pool.tile([64, 128], f32, tag="cd_pair")
                    nc.sync.dma_start(
                        out=cd_pair,
                        in_=cd_d[cc : cc + 128, :].rearrange(
                            "(p two) h -> p (two h)", two=2
                        ),
                    )
                    c_sq = prep_pool.tile([64, 128], f32, tag="c_sq")
                    nc.scalar.activation(
                        out=c_sq,
                        in_=cd_pair,
                        func=mybir.ActivationFunctionType.Sqrt,
                        scale=4.0,
                    )
                    c_rec = prep_pool.tile([64, 128], f32, tag="c_rec")
                    nc.vector.reciprocal(out=c_rec, in_=c_sq)
                    ps_c = ps_prep_pool.tile([128, 64], f32, tag="ps_prep")
                    nc.tensor.transpose(ps_c, c_rec, ident[0:64, 0:64])
                    c_pairs_cur = prep_pool.tile([128, 64], f32, tag="c_pairs")
                    nc.vector.tensor_copy(out=c_pairs_cur, in_=ps_c)

                if g % (64 // GROUP) == 0:
                    sc = b0
                    # s-pipe: s_b on partitions 64-127 (transposed via PE)
                    cd_nat = prep_pool.tile([64, 64], f32, tag="cd_nat")
                    nc.sync.dma_start(out=cd_nat, in_=cd_d[sc : sc + 64, :])
                    x2 = prep_pool.tile([64, 128], f32, tag="x2")
                    nc.gpsimd.memset(x2[:, 0:64], 0.0)
                    nc.scalar.activation(
                        out=x2[:, 64:128],
                        in_=cd_nat,
                        func=mybir.ActivationFunctionType.Sqrt,
                    )
                    ps_s = ps_prep_pool.tile([128, 64], f32, tag="ps_prep")
                    nc.tensor.transpose(ps_s, x2, ident[0:64, 0:64])
                    s_stack_cur = prep_pool.tile([128, 64], f32, tag="s_stack")
                    nc.vector.tensor_copy(
                        out=s_stack_cur[64:128, :], in_=ps_s[64:128, :]
                    )

                    # mu-pipe: [muhTE1; muh1] columns
                    mu_x = prep_pool.tile([64, 128], f32, tag="mu_x")
                    nc.sync.dma_start(out=mu_x[:, 0:64], in_=muhTE1_d[sc : sc + 64, :])
                    nc.sync.dma_start(out=mu_x[:, 64:128], in_=muh1_d[sc : sc + 64, :])
                    ps_u = ps_prep_pool.tile([128, 64], f32, tag="ps_prep")
                    nc.tensor.transpose(ps_u, mu_x, ident[0:64, 0:64])
                    mu_cols_cur = prep_pool.tile([128, 64], f32, tag="mu_cols")
                    nc.vector.tensor_copy(out=mu_cols_cur, in_=ps_u)

                # ---- main group of 8 batches ----
                # moving tile: rows 0-63 wt1, rows 64-127 wtTE1
                mv = mv_pool.tile([128, GROUP, NV], mmdt, tag="mv")
                nc.sync.dma_start(
                    out=mv[0:64, :, :],
                    in_=wt1_d[b0 : b0 + GROUP].rearrange("b h v -> h b v"),
                )
                nc.sync.dma_start(
                    out=mv[64:128, :, :],
                    in_=wtTE1_d[b0 : b0 + GROUP].rearrange("b h v -> h b v"),
                )

                # covh pair-stacked: partitions 0-63 = even batch, 64-127 = odd
                covh_raw = covh_pool.tile([128, NPAIR, NH], f32, tag="covh_raw")
                covh_view = covh_d[b0 : b0 + GROUP].rearrange(
                    "(p two) h v -> two h p v", two=2
                )
                nc.sync.dma_start(out=covh_raw[0:64, :, :], in_=covh_view[0])
                nc.sync.dma_start(out=covh_raw[64:128, :, :], in_=covh_view[1])

                # pre-scale covh rows by c_i = 0.5/s_i (per-partition scalar)
                covh_sc = covh_pool.tile([128, NPAIR, NH], f32, tag="covh_sc")
                gp0 = (g % (128 // GROUP)) * NPAIR
                for p in range(NPAIR):
                    nc.scalar.activation(
                        out=covh_sc[:, p, :],
                        in_=covh_raw[:, p, :],
                        func=mybir.ActivationFunctionType.Copy,
                        scale=c_pairs_cur[:, gp0 + p : gp0 + p + 1],
                    )

                stat = stat_pool.tile([128, GROUP, 65], mmdt, tag="stat")
                # covh^T blocks via PE transpose (two batches per transpose)
                for p in range(NPAIR):
                    ps_t = ps_tp_pool.tile([64, 128], f32, tag="ps_t")
                    nc.tensor.transpose(ps_t, covh_sc[:, p, :], ident)
                    nc.vector.tensor_copy(
                        out=stat[0:64, 2 * p : 2 * p + 2, 0:64],
                        in_=ps_t.rearrange("h (two i) -> h two i", two=2),
                    )

                lb = (g % (64 // GROUP)) * GROUP
                # diag(s) blocks on partitions 64-127
                for bi in range(GROUP):
                    nc.gpsimd.affine_select(
                        out=stat[64:128, bi, 0:64],
                        in_=s_stack_cur[64:128, lb + bi : lb + bi + 1].to_broadcast(
                            (64, 64)
                        ),
                        compare_op=mybir.AluOpType.is_equal,
                        fill=0.0,
                        base=0,
                        pattern=[[-1, 64]],
                        channel_multiplier=1,
                    )
                # mu column (col 64)
                nc.gpsimd.tensor_copy(
                    out=stat[:, :, 64:65],
                    in_=mu_cols_cur[:, lb : lb + GROUP].unsqueeze(-1),
                )

                # matmuls: one per batch, two batches share one PSUM bank
                ps_banks = []
                for p in range(NPAIR):
                    ps_b = ps_mm_pool.tile([65, 512], f32, tag="ps_mm")
                    ps_banks.append(ps_b)
                for bi in range(GROUP):
                    ps_b = ps_banks[bi // 2]
                    off = (bi % 2) * NV
                    nc.tensor.matmul(
                        ps_b[0:65, off : off + NV],
                        stat[:, bi, :],
                        mv[:, bi, :],
                        start=True,
                        stop=True,
                    )

                staging = stage_pool.tile([64, GROUP, NV], f32, tag="staging")
                rowbuf = row_pool.tile([65, GROUP * NV], f32, tag="rowbuf")
                for p in range(NPAIR):
                    nc.vector.tensor_copy(
                        out=staging[0:64, 2 * p : 2 * p + 2, :],
                        in_=ps_banks[p][0:64, :].rearrange(
                            "h (two v) -> h two v", two=2
                        ),
                    )
                    nc.scalar.activation(
                        out=rowbuf[64:65, p * 512 : (p + 1) * 512],
                        in_=ps_banks[p][64:65, 0:512],
                        func=mybir.ActivationFunctionType.Copy,
                    )

                nc.scalar.dma_start(
                    out=wtTE2_d[b0 : b0 + GROUP].rearrange("b h v -> h b v"),
                    in_=staging,
                )
                nc.scalar.dma_start(
                    out=scratch[b0 : b0 + GROUP, :],
                    in_=rowbuf[64:65, :],
                )

            # ---- final pass: bTE2 = scratch + bTE1 (two halves) ----
            for half in range(2):
                h0 = half * (BS // 2)
                h1 = h0 + BS // 2
                fin_in = fin_pool.tile([128, 2, NV], f32, tag="fin_in")
                nc.sync.dma_start(
                    out=fin_in,
                    in_=scratch[h0:h1, :].rearrange("(p two) v -> p two v", two=2),
                )
                fin_b1 = fin_pool.tile([128, 2, NV], f32, tag="fin_b1")
                nc.sync.dma_start(
                    out=fin_b1,
                    in_=bTE1_d[h0:h1, :].rearrange("(p two) v -> p two v", two=2),
                )
                fin_out = fin_pool.tile([128, 2, NV], f32, tag="fin_out")
                nc.vector.tensor_add(out=fin_out, in0=fin_in, in1=fin_b1)
                nc.scalar.dma_start(
                    out=bTE2_d[h0:h1, :].rearrange("(p two) v -> p two v", two=2),
                    in_=fin_out,
                )

    nc.finalize()
    return nc


def _get_program():
    if "nc" not in _cached:
        _cached["nc"] = _build_program()
    return _cached["nc"]


def _run(inputs, trace=False):
    nc = _get_program()
    in_maps = []
    for c in range(N_CORES):
        sl = slice(c * BS, (c + 1) * BS)
        in_maps.append(
            {
                "wtTE1": np.ascontiguousarray(inputs["wtTE1"][sl]),
                "wt1": np.ascontiguousarray(inputs["wt1"][sl]),
                "covhTE1": np.ascontiguousarray(inputs["covhTE1"][sl]),
                "covh_diag1": np.ascontiguousarray(inputs["covh_diag1"][sl]),
                "muh1": np.ascontiguousarray(inputs["muh1"][sl]),
                "muhTE1": np.ascontiguousarray(inputs["muhTE1"][sl]),
                "bTE1": np.ascontiguousarray(inputs["bTE1"][sl]),
            }
        )
    res = run_bass_kernel_spmd(
        nc, in_maps, list(range(N_CORES)), trace=trace
    )
    wtTE2 = np.concatenate([res.results[c]["wtTE2"] for c in range(N_CORES)], axis=0)
    bTE2 = np.concatenate([res.results[c]["bTE2"] for c in range(N_CORES)], axis=0)
    return (bTE2, wtTE2), res


def kernel(bTE1, wtTE1, muh1, wt1, muhTE1, covh_diag1, covhTE1, sig2TE):
    inputs = {
        "bTE1": np.asarray(bTE1, dtype=np.float32),
        "wtTE1": np.asarray(wtTE1, dtype=np.float32),
        "muh1": np.asarray(muh1, dtype=np.float32),
        "wt1": np.asarray(wt1, dtype=np.float32),
        "muhTE1": np.asarray(muhTE1, dtype=np.float32),
        "covh_diag1": np.asarray(covh_diag1, dtype=np.float32),
        "covhTE1": np.asarray(covhTE1, dtype=np.float32),
    }
    (bTE2, wtTE2), _ = _run(inputs, trace=False)
    return (np.asarray(sig2TE, dtype=np.float32), bTE2, wtTE2)
